# revision 1
# baseline (speedup 1.0000x reference)
"""Trainium2 Bass kernel for nn_DiffAttn (differential attention).

Reference computation (per batch b):
    Q = X @ Wq.T + bq ; K = X @ Wk.T + bk ; V = X @ Wv.T + bv
    Q1,Q2 / K1,K2 = halves of feature dim
    A_j = (Q_j @ K_j.T) / sqrt(DIM)
    out = softmax(A1) @ V - scalar * softmax(A2) @ V

Sharding: 8 cores = 4 batches x 2 query-halves. Each core computes the
full K/V projection for its batch (redundant within the pair) and the
attention output for its 1024 queries. No collectives needed; output
slabs are disjoint.

Device-side layouts avoid all on-chip transposes: the host pre-transposes
X^T and W^T so every matmul contraction dim lands on SBUF partitions.
Projection / score matmuls run in bf16; P=exp(scores) and V stay fp32
and the attention@V matmuls run as float32r (single-pass fp32, ~2
cycles/column). The attention weights are normalized BEFORE the V matmul
(A = P1/r1 - scalar*P2/r2) so only one attn@V GEMM is needed; row sums
come from an all-ones stationary matmul whose output is replicated
across partitions, and 1/r is computed as exp(-ln r) on the Scalar
engine. Measured on trn2: ~344 us HW exec, rel-err ~2.1e-3 vs the fp32
reference.
"""

import json
import math
import os
from contextlib import ExitStack

import numpy as np
import ml_dtypes

import concourse.bass as bass
import concourse.tile as tile
from concourse import mybir
from concourse.bass_utils import run_bass_kernel_spmd


def _split_waits(raw: bytes, max_waits: int = 1) -> bytes:
    """walrus's CoreV3 codegen rejects instructions carrying more than one
    sync wait ("Too many sync wait commands"); Tile's kernel-tail drain
    aggregates one wait per live processor. Hoist excess waits onto chained
    same-engine Drain instructions inserted immediately before the offender."""
    m = json.loads(raw)
    uid = 0
    for fn in m["functions"]:
        for blk in fn["blocks"]:
            out = []
            for ins in blk["instructions"]:
                sy = ins.get("sync_info") or {}
                waits = sy.get("on_wait") or []
                if len(waits) > max_waits:
                    head, keep = waits[:-max_waits], waits[-max_waits:]
                    while head:
                        chunk, head = head[:max_waits], head[max_waits:]
                        uid += 1
                        out.append(
                            {
                                "engine": ins["engine"],
                                "ins": [],
                                "is_reset_sema": False,
                                "name": f"{ins['name']}-wsplit{uid}",
                                "opcode": "Drain",
                                "outs": [],
                                "sync_info": {"on_update": [], "on_wait": chunk},
                            }
                        )
                    sy["on_wait"] = keep
                out.append(ins)
            blk["instructions"] = out
    return json.dumps(m).encode()

B, S, DIM = 4, 2048, 1024
H = DIM // 2
NCORES = 8
QLEN = S // 2          # queries per core
SCALE = 1.0 / math.sqrt(DIM)

BF16 = mybir.dt.bfloat16
F32 = mybir.dt.float32
F32R = mybir.dt.float32r

DT = DIM // 128        # 8  contraction tiles over model dim
CT = DIM // 128        # 8  feature tiles of Q^T/K^T
KT = S // 128          # 16 key tiles
NQC = QLEN // 512      # 2  query chunks of 512
VW = DIM              # V width (row sums come from an ones-row matmul instead)

# test harness hooks (the grader never touches these)
TRACE = False
LAST_RESULTS = None


def _build_bass():
    nc = bass.Bass(
        trn_type="TRN2",
        target_bir_lowering=False,
        debug=False,
        num_devices=NCORES,
    )

    xt = nc.dram_tensor("xt", [DIM, S], BF16, kind="ExternalInput")
    xtq = nc.dram_tensor("xtq", [DIM, QLEN], BF16, kind="ExternalInput")
    wqt = nc.dram_tensor("wqt", [DIM, DIM], BF16, kind="ExternalInput")
    wkt = nc.dram_tensor("wkt", [DIM, DIM], BF16, kind="ExternalInput")
    wvt = nc.dram_tensor("wvt", [DIM, DIM], BF16, kind="ExternalInput")
    bqr = nc.dram_tensor("bqr", [128, CT], F32, kind="ExternalInput")
    bkr = nc.dram_tensor("bkr", [128, CT], F32, kind="ExternalInput")
    bvb = nc.dram_tensor("bvb", [128, DIM], F32, kind="ExternalInput")
    scv = nc.dram_tensor("scv", [128, 1], F32, kind="ExternalInput")
    outp = nc.dram_tensor("out", [QLEN, DIM], F32, kind="ExternalOutput")

    Id = mybir.ActivationFunctionType.Identity
    Exp = mybir.ActivationFunctionType.Exp
    mult = mybir.AluOpType.mult
    subtract = mybir.AluOpType.subtract

    with tile.TileContext(nc) as tc, ExitStack() as ctx:
        const = ctx.enter_context(tc.tile_pool(name="const", bufs=1))
        persist = ctx.enter_context(tc.tile_pool(name="persist", bufs=1))
        ps_s = ctx.enter_context(
            tc.tile_pool(name="ps_s", bufs=3, space="PSUM")
        )

        bq_sb = const.tile([128, CT], F32)
        nc.sync.dma_start(out=bq_sb[:, :], in_=bqr[:, :])
        bk_sb = const.tile([128, CT], F32)
        nc.sync.dma_start(out=bk_sb[:, :], in_=bkr[:, :])
        sc_sb = const.tile([128, 1], F32)
        nc.sync.dma_start(out=sc_sb[:, :], in_=scv[:, :])
        ones_sb = const.tile([128, 2], F32)
        nc.vector.memset(ones_sb[:, :], 1.0)

        # Warm the PE clock gate (HAM) during the initial input-DMA wait:
        # a chain of tiny dependent matmuls gives ~4.5 us of sustained PE
        # activity so the first projection matmuls run at 2.4 GHz, not 1.2.
        with tc.psum_pool(name="ps_w", bufs=1) as ps_w:
            warm = ps_w.tile([2, 2], F32, name="warm")
            for _ in range(24):
                nc.tensor.matmul(
                    warm[:, :], ones_sb[:, :], ones_sb[:, :], start=True, stop=True
                )

        # persistent products of the projection phase
        q_sb = [persist.tile([128, QLEN], BF16, name=f"q{i}") for i in range(CT)]
        k_sb = [persist.tile([128, S], BF16, name=f"k{i}") for i in range(CT)]
        v_sb = [persist.tile([128, VW], F32R, name=f"v{i}") for i in range(KT)]

        # XT tiles live from before phase 1a through phase 1c (released below)
        xtp = tc.alloc_tile_pool(name="xtp", bufs=1)
        x_t = [xtp.tile([128, S], BF16, name=f"x{d}") for d in range(DT)]

        # wk prefetch pool outlives phase 1a (released after phase 1c)
        wkpre = tc.alloc_tile_pool(name="wkpre", bufs=1)
        wk_pre = [wkpre.tile([128, DIM], BF16, name=f"wkp{d}") for d in range(4)]

        # ---- Phase 1a: Q^T[c, q] = Wq^T.T @ X^T[:, qsel]  (+bq) ----
        with nc.named_scope("proj_q"), tc.tile_pool(name="wq", bufs=1) as wqp, tc.tile_pool(
            name="xq", bufs=1
        ) as xqp:
            wq_t = [wqp.tile([128, DIM], BF16, name=f"wq{d}") for d in range(DT)]
            xq_t = [xqp.tile([128, QLEN], BF16, name=f"xq{d}") for d in range(DT)]
            for d in range(DT):
                nc.sync.dma_start(out=xq_t[d][:, :], in_=xtq[d * 128 : (d + 1) * 128, :])
                nc.sync.dma_start(out=wq_t[d][:, :], in_=wqt[d * 128 : (d + 1) * 128, :])
            for d in range(DT):
                nc.sync.dma_start(out=x_t[d][:, :], in_=xt[d * 128 : (d + 1) * 128, :])
            for d in range(4):
                nc.sync.dma_start(out=wk_pre[d][:, :], in_=wkt[d * 128 : (d + 1) * 128, :])
            for c in range(CT):
                for n in range(QLEN // 512):
                    ps = ps_s.tile([128, 512], F32, tag="ps", name="psq")
                    for d in range(DT):
                        nc.tensor.matmul(
                            ps[:, :],
                            wq_t[d][:, c * 128 : (c + 1) * 128],
                            xq_t[d][:, n * 512 : (n + 1) * 512],
                            start=(d == 0),
                            stop=(d == DT - 1),
                        )
                    nc.scalar.activation(
                        q_sb[c][:, n * 512 : (n + 1) * 512],
                        ps[:, :],
                        Id,
                        bias=bq_sb[:, c : c + 1],
                    )

        # ---- Phase 1b: K^T[c, k] = Wk^T.T @ X^T  (+bk) ----
        with nc.named_scope("proj_kv"), tc.tile_pool(name="wk", bufs=1) as wkp:
            wk_t = wk_pre + [
                wkp.tile([128, DIM], BF16, name=f"wk{d}") for d in range(4, DT)
            ]
            for d in range(4, DT):
                nc.sync.dma_start(out=wk_t[d][:, :], in_=wkt[d * 128 : (d + 1) * 128, :])
            for c in range(CT):
                for n in range(S // 512):
                    ps = ps_s.tile([128, 512], F32, tag="ps", name="psk")
                    for d in range(DT):
                        nc.tensor.matmul(
                            ps[:, :],
                            wk_t[d][:, c * 128 : (c + 1) * 128],
                            x_t[d][:, n * 512 : (n + 1) * 512],
                            start=(d == 0),
                            stop=(d == DT - 1),
                        )
                    nc.scalar.activation(
                        k_sb[c][:, n * 512 : (n + 1) * 512],
                        ps[:, :],
                        Id,
                        bias=bk_sb[:, c : c + 1],
                    )

            # ---- Phase 1c: V[k, d] = X^T.T @ Wv^T  (+bv broadcast) ----
            # x_t (X^T tiles) stay resident as the stationary operand.
            with tc.tile_pool(name="wv", bufs=1) as wvp:
                bv_sb = wvp.tile([128, DIM], F32, name="bv_sb")
                nc.sync.dma_start(out=bv_sb[:, :], in_=bvb[:, :])
                wv_t = [wvp.tile([128, DIM], BF16, name=f"wv{d}") for d in range(DT)]
                for d in range(DT):
                    nc.sync.dma_start(
                        out=wv_t[d][:, :], in_=wvt[d * 128 : (d + 1) * 128, :]
                    )
                for k in range(KT):
                    for n in range(DIM // 512):
                        ps = ps_s.tile([128, 512], F32, tag="ps", name="psv")
                        for d in range(DT):
                            nc.tensor.matmul(
                                ps[:, :],
                                x_t[d][:, k * 128 : (k + 1) * 128],
                                wv_t[d][:, n * 512 : (n + 1) * 512],
                                start=(d == 0),
                                stop=(d == DT - 1),
                            )
                        nc.vector.tensor_add(
                            v_sb[k][:, n * 512 : (n + 1) * 512],
                            ps[:, :],
                            bv_sb[:, n * 512 : (n + 1) * 512],
                        )

        wkpre.release()
        xtp.release()

        # ---- Phase 2: attention, one 512-query chunk at a time ----
        # Normalize P before the V matmul so only ONE attn@V GEMM is needed:
        #   A^T = P1^T * bcast(1/r1) - P2^T * bcast(scalar/r2);  out = A^T.T @ V
        # r_j comes from an ones-row stationary matmul (column sums of P^T);
        # bcast replicates the [1, q] reciprocal row across partitions via a
        # K=1 ones-column matmul.
        lnsc_sb = const.tile([128, 1], F32)
        nc.scalar.activation(lnsc_sb[:, :], sc_sb[:, :], mybir.ActivationFunctionType.Ln)
        ones_sq = const.tile([128, 128], F32R)
        ones_sqf = const.tile([128, 128], F32)
        nc.vector.memset(ones_sqf[:, :], 1.0)
        nc.vector.tensor_copy(ones_sq[:, :], ones_sqf[:, :])

        with (
            tc.tile_pool(name="pP", bufs=1) as pP,
            tc.tile_pool(name="ps_r", bufs=1, space="PSUM") as ps_r,
            tc.tile_pool(name="ps_u", bufs=4, space="PSUM") as ps_u,
            tc.tile_pool(name="small", bufs=4) as small,
            tc.tile_pool(name="tmp2", bufs=2) as tmp2,
            tc.tile_pool(name="ostage", bufs=2) as ostage,
        ):
            p_sb = [
                [pP.tile([128, 512], F32R, name=f"p{j}_{k}") for k in range(KT)]
                for j in range(2)
            ]
            for qc in range(NQC):
                # scores S^T[k, q] = K_j^T.T @ Q_j^T; P = exp(s*S^T); r = col sums
                bcs = []
                scope_s = nc.enter_named_scope(f"attn_s{qc}", False)
                for j in range(2):
                    # r replicated across partitions: ones[128,128].T @ P = col sums
                    r_ps = ps_r.tile([128, 512], F32, tag="r", name=f"r{j}")
                    for k in range(KT):
                        ps = ps_s.tile([128, 512], F32, tag="ps", name="pss")
                        for ci in range(4):
                            c = 4 * j + ci
                            nc.tensor.matmul(
                                ps[:, :],
                                k_sb[c][:, k * 128 : (k + 1) * 128],
                                q_sb[c][:, qc * 512 : (qc + 1) * 512],
                                start=(ci == 0),
                                stop=(ci == 3),
                            )
                        nc.scalar.activation(
                            p_sb[j][k][:, :], ps[:, :], Exp, scale=SCALE
                        )
                        nc.tensor.matmul(
                            r_ps[:, :],
                            ones_sq[:, :],
                            p_sb[j][k][:, :],
                            start=(k == 0),
                            stop=(k == KT - 1),
                        )
                    # bc_j = exp(-ln r_j) = 1/r_j on the Scalar engine
                    # (j=1 folds the input scalar in via a +ln(scalar) bias)
                    lnr = tmp2.tile([128, 512], F32, tag="lnr", name="lnr")
                    nc.scalar.activation(
                        lnr[:, :], r_ps[:, :], mybir.ActivationFunctionType.Ln
                    )
                    bc = small.tile([128, 512], F32, tag=f"bc{j}", name=f"bc{j}")
                    if j == 0:
                        nc.scalar.activation(bc[:, :], lnr[:, :], Exp, scale=-1.0)
                    else:
                        nc.scalar.activation(
                            bc[:, :], lnr[:, :], Exp, scale=-1.0, bias=lnsc_sb[:, :]
                        )
                    bcs.append(bc)
                nc.leave_named_scope(f"attn_s{qc}", scope_s[0], False)

                # A^T[k] = P1[k]*bc1 - P2[k]*bc2s  (in place into p_sb[0])
                scope_a = nc.enter_named_scope(f"attn_a{qc}", False)
                for k in range(KT):
                    t2 = tmp2.tile([128, 512], F32, tag="t2", name="t2")
                    nc.vector.tensor_mul(t2[:, :], p_sb[0][k][:, :], bcs[0][:, :])
                    nc.vector.tensor_mul(
                        p_sb[1][k][:, :], p_sb[1][k][:, :], bcs[1][:, :]
                    )
                    nc.vector.tensor_sub(p_sb[1][k][:, :], t2[:, :], p_sb[1][k][:, :])
                nc.leave_named_scope(f"attn_a{qc}", scope_a[0], False)

                # out rows = A^T.T @ V
                scope_u = nc.enter_named_scope(f"attn_u{qc}", False)
                for t in range(4):
                    row = qc * 512 + t * 128
                    for n in range(DIM // 512):
                        lo, hi = n * 512, (n + 1) * 512
                        u = ps_u.tile([128, 512], F32, tag="u", name="u")
                        for k in range(KT):
                            nc.tensor.matmul(
                                u[:, :],
                                p_sb[1][k][:, t * 128 : (t + 1) * 128],
                                v_sb[k][:, lo:hi],
                                start=(k == 0),
                                stop=(k == KT - 1),
                            )
                        o = ostage.tile([128, 512], F32, tag="o", name="o")
                        if n == 0:
                            nc.scalar.copy(o[:, :], u[:, :])
                        else:
                            nc.vector.tensor_copy(o[:, :], u[:, :])
                        nc.sync.dma_start(
                            out=outp[row : row + 128, lo:hi], in_=o[:, :]
                        )
                nc.leave_named_scope(f"attn_u{qc}", scope_u[0], False)

    return nc


_NC_CACHE = None


def _get_nc():
    global _NC_CACHE
    if _NC_CACHE is None:
        nc = _build_bass()
        fixed = _split_waits(bass.Bass.to_json_bytes(nc))
        nc.to_json_bytes = lambda: fixed
        _NC_CACHE = nc
    return _NC_CACHE


def kernel(hidden_states, W_q, b_q, W_k, b_k, W_v, b_v, scalar):
    global LAST_RESULTS
    bf16 = ml_dtypes.bfloat16
    X = np.asarray(hidden_states, np.float32)
    wqt = np.ascontiguousarray(np.asarray(W_q, np.float32).T).astype(bf16)
    wkt = np.ascontiguousarray(np.asarray(W_k, np.float32).T).astype(bf16)
    wvt = np.ascontiguousarray(np.asarray(W_v, np.float32).T).astype(bf16)
    bqr = np.ascontiguousarray(np.asarray(b_q, np.float32).reshape(CT, 128).T)
    bkr = np.ascontiguousarray(np.asarray(b_k, np.float32).reshape(CT, 128).T)
    bvb = np.ascontiguousarray(
        np.broadcast_to(np.asarray(b_v, np.float32), (128, DIM))
    )
    scv = np.full((128, 1), np.asarray(scalar, np.float32).reshape(-1)[0], np.float32)

    in_maps = []
    xts = {}
    for core in range(NCORES):
        b, h = core // 2, core % 2
        if b not in xts:
            xts[b] = np.ascontiguousarray(X[b].T).astype(bf16)
        xt_b = xts[b]
        xtq = np.ascontiguousarray(xt_b[:, h * QLEN : (h + 1) * QLEN])
        in_maps.append(
            {
                "xt": xt_b,
                "xtq": xtq,
                "wqt": wqt,
                "wkt": wkt,
                "wvt": wvt,
                "bqr": bqr,
                "bkr": bkr,
                "bvb": bvb,
                "scv": scv,
            }
        )

    nc = _get_nc()
    res = run_bass_kernel_spmd(
        nc,
        in_maps,
        list(range(NCORES)),
        trace=TRACE,
    )
    LAST_RESULTS = res

    out = np.empty((B, S, DIM), np.float32)
    for core in range(NCORES):
        b, h = core // 2, core % 2
        out[b, h * QLEN : (h + 1) * QLEN, :] = res.results[core]["out"]
    return out


if __name__ == "__main__":
    import reference

    inputs = {k: np.asarray(v) for k, v in reference.setup_inputs().items()}
    got = kernel(**inputs)
    print("kernel output", got.shape, got.dtype)



# revision 5
# speedup vs baseline: 1.0833x; 1.0833x over previous
"""Trainium2 Bass kernel for nn_DiffAttn (differential attention).

Reference computation (per batch b):
    Q = X @ Wq.T + bq ; K = X @ Wk.T + bk ; V = X @ Wv.T + bv
    Q1,Q2 / K1,K2 = halves of feature dim
    A_j = (Q_j @ K_j.T) / sqrt(DIM)
    out = softmax(A1) @ V - scalar * softmax(A2) @ V

Sharding: 8 cores = 4 batches x 2 query-halves. Each core computes the
full K/V projection for its batch (redundant within the pair) and the
attention output for its 1024 queries. No collectives needed; output
slabs are disjoint.

v2 over the 344us baseline:
  - everything bf16 on the PE: P=exp(scores), V, the ones vector and the
    attention matrix all run 1 cycle/column (fp32r measured 2 cyc/col on
    HW -> ~68us of tensor time back).
  - host pre-packs each input as a single SBUF-image so each tensor is
    ONE DMA_DIRECT2D (each trigger costs ~600ns of sync-queue issue;
    the baseline's 28 input DMAs serialized ~17us of loading).
  - both query-chunks' scores are emitted before either attn@V so the
    DVE normalize pass never stalls the PE.
  - output is staged [128, 4096] per query-chunk and written as 8 big
    DMAs on the idle gpsimd queue; host untiles.
"""

import json
import math
from contextlib import ExitStack

import numpy as np
import ml_dtypes

import concourse.bass as bass
import concourse.tile as tile
from concourse import mybir
from concourse.bass_utils import run_bass_kernel_spmd


def _split_waits(raw: bytes, max_waits: int = 1) -> bytes:
    """walrus's CoreV3 codegen rejects instructions carrying more than one
    sync wait ("Too many sync wait commands"); Tile's kernel-tail drain
    aggregates one wait per live processor. Hoist excess waits onto chained
    same-engine Drain instructions inserted immediately before the offender."""
    m = json.loads(raw)
    uid = 0
    for fn in m["functions"]:
        for blk in fn["blocks"]:
            out = []
            for ins in blk["instructions"]:
                sy = ins.get("sync_info") or {}
                waits = sy.get("on_wait") or []
                if len(waits) > max_waits:
                    head, keep = waits[:-max_waits], waits[-max_waits:]
                    while head:
                        chunk, head = head[:max_waits], head[max_waits:]
                        uid += 1
                        out.append(
                            {
                                "engine": ins["engine"],
                                "ins": [],
                                "is_reset_sema": False,
                                "name": f"{ins['name']}-wsplit{uid}",
                                "opcode": "Drain",
                                "outs": [],
                                "sync_info": {"on_update": [], "on_wait": chunk},
                            }
                        )
                    sy["on_wait"] = keep
                out.append(ins)
            blk["instructions"] = out
    return json.dumps(m).encode()


B, S, DIM = 4, 2048, 1024
H = DIM // 2
NCORES = 8
QLEN = S // 2          # queries per core
SCALE = 1.0 / math.sqrt(DIM)

BF16 = mybir.dt.bfloat16
F32 = mybir.dt.float32

DT = DIM // 128        # 8  contraction tiles over model dim
CT = DIM // 128        # 8  feature tiles of Q^T/K^T
KT = S // 128          # 16 key tiles
NQC = QLEN // 512      # 2  query chunks of 512
NCST = 1 + CT + CT + DIM  # sc | bq | bk | bv

# test harness hooks (the grader never touches these)
TRACE = False
LAST_RESULTS = None


def _build_bass():
    nc = bass.Bass(
        trn_type="TRN2",
        target_bir_lowering=False,
        debug=False,
        num_devices=NCORES,
    )

    xti = nc.dram_tensor("xti", [128, DT * S], BF16, kind="ExternalInput")
    xqi = nc.dram_tensor("xqi", [128, DT * QLEN], BF16, kind="ExternalInput")
    wqi = nc.dram_tensor("wqi", [128, DT * DIM], BF16, kind="ExternalInput")
    wki = nc.dram_tensor("wki", [128, DT * DIM], BF16, kind="ExternalInput")
    wvi = nc.dram_tensor("wvi", [128, DT * DIM], BF16, kind="ExternalInput")
    cst = nc.dram_tensor("cst", [128, NCST], F32, kind="ExternalInput")
    outp = nc.dram_tensor("out", [NQC * 128, 4096], F32, kind="ExternalOutput")

    Id = mybir.ActivationFunctionType.Identity
    Exp = mybir.ActivationFunctionType.Exp
    Ln = mybir.ActivationFunctionType.Ln

    with tile.TileContext(nc) as tc, ExitStack() as ctx:
        const = ctx.enter_context(tc.tile_pool(name="const", bufs=1))
        persist = ctx.enter_context(tc.tile_pool(name="persist", bufs=1))
        ps_s = ctx.enter_context(
            tc.tile_pool(name="ps_s", bufs=4, space="PSUM")
        )

        cst_sb = const.tile([128, NCST], F32)
        nc.sync.dma_start(out=cst_sb[:, :], in_=cst[:, :])
        sc_sb = cst_sb[:, 0:1]
        bq_sb = cst_sb[:, 1 : 1 + CT]
        bk_sb = cst_sb[:, 1 + CT : 1 + 2 * CT]
        bv_sb = cst_sb[:, 1 + 2 * CT : 1 + 2 * CT + DIM]

        ones_w = const.tile([128, 128], BF16)
        nc.vector.memset(ones_w[:, :], 1.0)
        ones_m = const.tile([128, 512], BF16)
        nc.vector.memset(ones_m[:, :], 1.0)

        # phase-scoped input images; tile pools release LIFO, so allocate
        # in reverse of release order (wq+xq after Q, wk after K, xt+wv after V)
        wvp = tc.alloc_tile_pool(name="wvp", bufs=1)
        wv_im = [wvp.tile([128, DT * DIM // 2], BF16, name=f"wv{h}") for h in range(2)]
        xtp = tc.alloc_tile_pool(name="xtp", bufs=1)
        xt_im = [xtp.tile([128, DT * S // 2], BF16, name=f"xt{h}") for h in range(2)]
        wkp = tc.alloc_tile_pool(name="wkp", bufs=1)
        wk_im = [wkp.tile([128, DT * DIM // 2], BF16, name=f"wk{h}") for h in range(2)]
        xqp = tc.alloc_tile_pool(name="xqp", bufs=1)
        xq_im = [xqp.tile([128, DT * QLEN // 2], BF16, name=f"xq{h}") for h in range(2)]
        wqp = tc.alloc_tile_pool(name="wqp", bufs=1)
        wq_im = [wqp.tile([128, DT * DIM // 2], BF16, name=f"wq{h}") for h in range(2)]

        HW_ = DT * DIM // 2  # columns per half-image (4096)
        HX = DT * S // 2     # 8192
        HQ = DT * QLEN // 2  # 4096

        # Issue order matters: each DMA trigger is ~600ns of sync-queue
        # time and phases consume tensors in this order.
        nc.sync.dma_start(out=wq_im[0][:, :], in_=wqi[:, 0:HW_])
        nc.sync.dma_start(out=xq_im[0][:, :], in_=xqi[:, 0:HQ])
        nc.sync.dma_start(out=wq_im[1][:, :], in_=wqi[:, HW_:])
        nc.sync.dma_start(out=xq_im[1][:, :], in_=xqi[:, HQ:])
        nc.sync.dma_start(out=wk_im[0][:, :], in_=wki[:, 0:HW_])
        nc.sync.dma_start(out=xt_im[0][:, :], in_=xti[:, 0:HX])
        nc.sync.dma_start(out=wk_im[1][:, :], in_=wki[:, HW_:])
        nc.sync.dma_start(out=xt_im[1][:, :], in_=xti[:, HX:])
        nc.sync.dma_start(out=wv_im[0][:, :], in_=wvi[:, 0:HW_])
        nc.sync.dma_start(out=wv_im[1][:, :], in_=wvi[:, HW_:])

        def wsl(im, d, lo, hi):
            """slice [d*128:(d+1)*128, lo:hi] of a [1024, 1024]-like image"""
            h, dd = divmod(d, DT // 2)
            return im[h][:, dd * DIM + lo : dd * DIM + hi]

        def xsl(d, lo, hi):
            h, dd = divmod(d, DT // 2)
            return xt_im[h][:, dd * S + lo : dd * S + hi]

        def qsl(d, lo, hi):
            h, dd = divmod(d, DT // 2)
            return xq_im[h][:, dd * QLEN + lo : dd * QLEN + hi]

        # Warm the PE clock gate (HAM) during the initial input-DMA wait:
        # sustained PE activity so the first projection matmuls run at
        # 2.4 GHz, not 1.2 (needs ~3.4us of busy to flip the SHORT window).
        with tc.psum_pool(name="ps_w", bufs=1) as ps_w:
            warm = ps_w.tile([128, 512], F32, name="warm")
            for i in range(12):
                nc.tensor.matmul(
                    warm[:, :], ones_w[:, :], ones_m[:, :], start=(i == 0), stop=(i == 11)
                )

        # persistent products of the projection phase
        q_sb = [persist.tile([128, QLEN], BF16, name=f"q{i}") for i in range(CT)]
        k_sb = [persist.tile([128, S], BF16, name=f"k{i}") for i in range(CT)]
        v_sb = [persist.tile([128, DIM], BF16, name=f"v{i}") for i in range(KT)]

        # ---- Phase 1a: Q^T[c, q] = Wq^T.T @ X^T[:, qsel]  (+bq) ----
        with nc.named_scope("proj_q"):
            for c in range(CT):
                for half in range(QLEN // 1024):
                    pss = [ps_s.tile([128, 512], F32, tag="ps", name="psq") for _ in range(2)]
                    for d in range(DT):
                        for n in range(2):
                            q0 = half * 1024 + n * 512
                            nc.tensor.matmul(
                                pss[n][:, :],
                                wsl(wq_im, d, c * 128, (c + 1) * 128),
                                qsl(d, q0, q0 + 512),
                                start=(d == 0),
                                stop=(d == DT - 1),
                            )
                    for n in range(2):
                        q0 = half * 1024 + n * 512
                        nc.scalar.activation(
                            q_sb[c][:, q0 : q0 + 512],
                            pss[n][:, :],
                            Id,
                            bias=bq_sb[:, c : c + 1],
                        )

        wqp.release()
        xqp.release()

        # ---- Phase 1b: K^T[c, k] = Wk^T.T @ X^T  (+bk) ----
        with nc.named_scope("proj_k"):
            for c in range(CT):
                for half in range(S // 1024):
                    pss = [ps_s.tile([128, 512], F32, tag="ps", name="psk") for _ in range(2)]
                    for d in range(DT):
                        for n in range(2):
                            s0 = half * 1024 + n * 512
                            nc.tensor.matmul(
                                pss[n][:, :],
                                wsl(wk_im, d, c * 128, (c + 1) * 128),
                                xsl(d, s0, s0 + 512),
                                start=(d == 0),
                                stop=(d == DT - 1),
                            )
                    for n in range(2):
                        s0 = half * 1024 + n * 512
                        nc.scalar.activation(
                            k_sb[c][:, s0 : s0 + 512],
                            pss[n][:, :],
                            Id,
                            bias=bk_sb[:, c : c + 1],
                        )

        wkp.release()  # (xqp, wqp already popped; wkp is now top of stack)

        # ---- Phase 1c: V[k, d] = X^T.T @ Wv^T  (+bv broadcast) ----
        with nc.named_scope("proj_v"):
            for k in range(KT):
                pss = [ps_s.tile([128, 512], F32, tag="ps", name="psv") for _ in range(2)]
                for d in range(DT):
                    for n in range(2):
                        nc.tensor.matmul(
                            pss[n][:, :],
                            xsl(d, k * 128, (k + 1) * 128),
                            wsl(wv_im, d, n * 512, (n + 1) * 512),
                            start=(d == 0),
                            stop=(d == DT - 1),
                        )
                for n in range(2):
                    nc.vector.tensor_add(
                        v_sb[k][:, n * 512 : (n + 1) * 512],
                        pss[n][:, :],
                        bv_sb[:, n * 512 : (n + 1) * 512],
                    )

        xtp.release()
        wvp.release()

        # ---- Phase 2: attention ----
        # Normalize P before the V matmul so only ONE attn@V GEMM is needed:
        #   A^T = P1^T * bcast(1/r1) - P2^T * bcast(scalar/r2);  out = A^T.T @ V
        # r_j comes from an ones-row stationary matmul (output replicated
        # across partitions); 1/r = exp(-ln r) on the Scalar engine (j=1
        # folds the input scalar in via a +ln(scalar) bias).
        # BOTH query-chunks' scores are emitted before either attn@V so the
        # PE never waits on the DVE normalize.
        lnsc_sb = const.tile([128, 1], F32)
        nc.scalar.activation(lnsc_sb[:, :], sc_sb, Ln)

        with (
            tc.tile_pool(name="pP", bufs=1) as pP,
            tc.tile_pool(name="ps_r", bufs=2, space="PSUM") as ps_r,
            tc.tile_pool(name="ps_u", bufs=2, space="PSUM") as ps_u,
            tc.tile_pool(name="small", bufs=8) as small,
            tc.tile_pool(name="tmp2", bufs=3) as tmp2,
            tc.tile_pool(name="ostage", bufs=3) as ostage,
        ):
            p_sb = [
                [
                    [pP.tile([128, 512], BF16, name=f"p{qc}_{j}_{k}") for k in range(KT)]
                    for j in range(2)
                ]
                for qc in range(NQC)
            ]
            bcs = [[None, None] for _ in range(NQC)]

            # scores + row-sums for all chunks first
            for qc in range(NQC):
                scope_s = nc.enter_named_scope(f"attn_s{qc}", False)
                for j in range(2):
                    r_ps = ps_r.tile([128, 512], F32, tag="r", name=f"r{qc}{j}")
                    for k in range(KT):
                        ps = ps_s.tile([128, 512], F32, tag="ps", name="pss")
                        for ci in range(4):
                            c = 4 * j + ci
                            nc.tensor.matmul(
                                ps[:, :],
                                k_sb[c][:, k * 128 : (k + 1) * 128],
                                q_sb[c][:, qc * 512 : (qc + 1) * 512],
                                start=(ci == 0),
                                stop=(ci == 3),
                            )
                        nc.scalar.activation(
                            p_sb[qc][j][k][:, :], ps[:, :], Exp, scale=SCALE
                        )
                        nc.tensor.matmul(
                            r_ps[:, :],
                            ones_w[:, :],
                            p_sb[qc][j][k][:, :],
                            start=(k == 0),
                            stop=(k == KT - 1),
                        )
                    lnr = tmp2.tile([128, 512], F32, tag="lnr", name="lnr")
                    nc.scalar.activation(lnr[:, :], r_ps[:, :], Ln)
                    bc = small.tile([128, 512], BF16, tag=f"bc{qc}{j}", name=f"bc{qc}{j}")
                    if j == 0:
                        nc.scalar.activation(bc[:, :], lnr[:, :], Exp, scale=-1.0)
                    else:
                        nc.scalar.activation(
                            bc[:, :], lnr[:, :], Exp, scale=-1.0, bias=lnsc_sb[:, :]
                        )
                    bcs[qc][j] = bc
                nc.leave_named_scope(f"attn_s{qc}", scope_s[0], False)

            for qc in range(NQC):
                # A^T[k] = P1[k]*bc1 - P2[k]*bc2s  (in place into p_sb[qc][1])
                scope_a = nc.enter_named_scope(f"attn_a{qc}", False)
                for k in range(KT):
                    t2 = tmp2.tile([128, 512], BF16, tag="t2", name="t2")
                    nc.vector.tensor_mul(t2[:, :], p_sb[qc][0][k][:, :], bcs[qc][0][:, :])
                    nc.vector.tensor_mul(
                        p_sb[qc][1][k][:, :], p_sb[qc][1][k][:, :], bcs[qc][1][:, :]
                    )
                    nc.vector.tensor_sub(
                        p_sb[qc][1][k][:, :], t2[:, :], p_sb[qc][1][k][:, :]
                    )
                nc.leave_named_scope(f"attn_a{qc}", scope_a[0], False)

                # out rows = A^T.T @ V ; stage [128, 1024] per t, DMA on gpsimd
                scope_u = nc.enter_named_scope(f"attn_u{qc}", False)
                for t in range(4):
                    us = [ps_u.tile([128, 512], F32, tag="u", name="u") for _ in range(2)]
                    for k in range(KT):
                        for n in range(2):
                            nc.tensor.matmul(
                                us[n][:, :],
                                p_sb[qc][1][k][:, t * 128 : (t + 1) * 128],
                                v_sb[k][:, n * 512 : (n + 1) * 512],
                                start=(k == 0),
                                stop=(k == KT - 1),
                            )
                    o = ostage.tile([128, 1024], F32, tag="o", name="o")
                    nc.scalar.copy(o[:, 0:512], us[0][:, :])
                    nc.vector.tensor_copy(o[:, 512:1024], us[1][:, :])
                    nc.gpsimd.dma_start(
                        out=outp[qc * 128 : (qc + 1) * 128, t * 1024 : (t + 1) * 1024],
                        in_=o[:, :],
                    )
                nc.leave_named_scope(f"attn_u{qc}", scope_u[0], False)

    return nc


_NC_CACHE = None


def _get_nc():
    global _NC_CACHE
    if _NC_CACHE is None:
        nc = _build_bass()
        fixed = _split_waits(bass.Bass.to_json_bytes(nc))
        nc.to_json_bytes = lambda: fixed
        _NC_CACHE = nc
    return _NC_CACHE


def _img(a32):
    """[1024, W] fp32 -> [128, 8*W] bf16 SBUF image (d-major blocks)."""
    W = a32.shape[1]
    return np.ascontiguousarray(
        a32.reshape(DT, 128, W).transpose(1, 0, 2).reshape(128, DT * W)
    ).astype(ml_dtypes.bfloat16)


def kernel(hidden_states, W_q, b_q, W_k, b_k, W_v, b_v, scalar):
    global LAST_RESULTS
    X = np.asarray(hidden_states, np.float32)
    wq_img = _img(np.ascontiguousarray(np.asarray(W_q, np.float32).T))
    wk_img = _img(np.ascontiguousarray(np.asarray(W_k, np.float32).T))
    wv_img = _img(np.ascontiguousarray(np.asarray(W_v, np.float32).T))

    cst = np.empty((128, NCST), np.float32)
    cst[:, 0] = np.asarray(scalar, np.float32).reshape(-1)[0]
    cst[:, 1 : 1 + CT] = np.asarray(b_q, np.float32).reshape(CT, 128).T
    cst[:, 1 + CT : 1 + 2 * CT] = np.asarray(b_k, np.float32).reshape(CT, 128).T
    cst[:, 1 + 2 * CT :] = np.broadcast_to(np.asarray(b_v, np.float32), (128, DIM))

    in_maps = []
    xt_imgs = {}
    for core in range(NCORES):
        b, h = core // 2, core % 2
        if b not in xt_imgs:
            xt_imgs[b] = _img(np.ascontiguousarray(X[b].T))
        xt_img = xt_imgs[b]
        # query-half image: columns h*1024..(h+1)*1024 of each d-block
        xq_img = np.ascontiguousarray(
            xt_img.reshape(128, DT, S)[:, :, h * QLEN : (h + 1) * QLEN].reshape(
                128, DT * QLEN
            )
        )
        in_maps.append(
            {
                "xti": xt_img,
                "xqi": xq_img,
                "wqi": wq_img,
                "wki": wk_img,
                "wvi": wv_img,
                "cst": cst,
            }
        )

    nc = _get_nc()
    res = run_bass_kernel_spmd(
        nc,
        in_maps,
        list(range(NCORES)),
        trace=TRACE,
    )
    LAST_RESULTS = res

    out = np.empty((B, S, DIM), np.float32)
    for core in range(NCORES):
        b, h = core // 2, core % 2
        # device layout [qc*128+p, t*1024 + n*512 + cc] -> [qc*512+t*128+p, :]
        dev = res.results[core]["out"].reshape(NQC, 128, 4, DIM)
        out[b, h * QLEN : (h + 1) * QLEN, :] = (
            dev.transpose(0, 2, 1, 3).reshape(QLEN, DIM)
        )
    return out


if __name__ == "__main__":
    import reference

    inputs = {k: np.asarray(v) for k, v in reference.setup_inputs().items()}
    got = kernel(**inputs)
    print("kernel output", got.shape, got.dtype)


# revision 6
# speedup vs baseline: 1.3131x; 1.2121x over previous
"""Trainium2 Bass kernel for nn_DiffAttn (differential attention).

Reference computation (per batch b):
    Q = X @ Wq.T + bq ; K = X @ Wk.T + bk ; V = X @ Wv.T + bv
    Q1,Q2 / K1,K2 = halves of feature dim
    A_j = (Q_j @ K_j.T) / sqrt(DIM)
    out = softmax(A1) @ V - scalar * softmax(A2) @ V

Sharding: 8 cores = 4 batches x 2 query-halves. Each core projects Q for
its own 1024 queries and K/V for its own 1024 KEYS (keys-half == query-
half, so the only X the core ever touches is its local [1024, 1024]
slice). The K^T and V halves are then exchanged within each batch-pair
via 2-rank AllGathers (groups [0,1][2,3][4,5][6,7]), chunked in 1MB
pieces so the wire time hides under the projection/scores matmuls.
AllGather concatenates by rank = by global key index, so the gathered
k_full/v_full are indexed identically on both cores of a pair (the SPMD
program never needs to know its own parity). Attention consumes only the
gathered copies.

Everything on the PE is bf16 (fp32r measured 2 cyc/col vs 1 for bf16);
accumulation fp32 in PSUM. P is normalized before the single attn@V GEMM:
A^T = P1^T*(1/r1) - P2^T*(scalar/r2), r from an ones-row matmul, 1/r =
exp(-ln r) on the Scalar engine. Both query-chunks' scores are emitted
before either attn@V so the DVE normalize never stalls the PE. Inputs
are host-packed SBUF images (one ~600ns DMA trigger per tensor instead
of 28); output is staged per 512-column chunk and written on the gpsimd
queue.
"""

import json
import math
from contextlib import ExitStack

import numpy as np
import ml_dtypes

import concourse.bass as bass
import concourse.tile as tile
from concourse import mybir
from concourse.bass_utils import run_bass_kernel_spmd


def _split_waits(raw: bytes, max_waits: int = 1) -> bytes:
    """walrus's CoreV3 codegen rejects instructions carrying more than one
    sync wait ("Too many sync wait commands"); Tile's kernel-tail drain
    aggregates one wait per live processor. Hoist excess waits onto chained
    same-engine Drain instructions inserted immediately before the offender."""
    m = json.loads(raw)
    uid = 0
    for fn in m["functions"]:
        for blk in fn["blocks"]:
            out = []
            for ins in blk["instructions"]:
                sy = ins.get("sync_info") or {}
                waits = sy.get("on_wait") or []
                if len(waits) > max_waits:
                    head, keep = waits[:-max_waits], waits[-max_waits:]
                    while head:
                        chunk, head = head[:max_waits], head[max_waits:]
                        uid += 1
                        out.append(
                            {
                                "engine": ins["engine"],
                                "ins": [],
                                "is_reset_sema": False,
                                "name": f"{ins['name']}-wsplit{uid}",
                                "opcode": "Drain",
                                "outs": [],
                                "sync_info": {"on_update": [], "on_wait": chunk},
                            }
                        )
                    sy["on_wait"] = keep
                out.append(ins)
            blk["instructions"] = out
    return json.dumps(m).encode()


B, S, DIM = 4, 2048, 1024
H = DIM // 2
NCORES = 8
QLEN = S // 2          # queries (== local keys) per core
SCALE = 1.0 / math.sqrt(DIM)

BF16 = mybir.dt.bfloat16
F32 = mybir.dt.float32

DT = DIM // 128        # 8  contraction tiles over model dim
CT = DIM // 128        # 8  feature tiles of Q^T/K^T
KT = S // 128          # 16 key tiles (global)
LKT = QLEN // 128      # 8  local key tiles
NQC = QLEN // 512      # 2  query chunks of 512
NCST = 1 + CT + CT + DIM  # sc | bq | bk | bv
GROUPS = [[0, 1], [2, 3], [4, 5], [6, 7]]

# test harness hooks (the grader never touches these)
TRACE = False
LAST_RESULTS = None


def _build_bass():
    nc = bass.Bass(
        trn_type="TRN2",
        target_bir_lowering=False,
        debug=False,
        num_devices=NCORES,
    )

    xqi = nc.dram_tensor("xqi", [128, DT * QLEN], BF16, kind="ExternalInput")
    wqi = nc.dram_tensor("wqi", [128, DT * DIM], BF16, kind="ExternalInput")
    wki = nc.dram_tensor("wki", [128, DT * DIM], BF16, kind="ExternalInput")
    wvi = nc.dram_tensor("wvi", [128, DT * DIM], BF16, kind="ExternalInput")
    cst = nc.dram_tensor("cst", [128, NCST], F32, kind="ExternalInput")
    outp = nc.dram_tensor("out", [NQC * 128, 4096], F32, kind="ExternalOutput")

    Id = mybir.ActivationFunctionType.Identity
    Exp = mybir.ActivationFunctionType.Exp
    Ln = mybir.ActivationFunctionType.Ln

    with tile.TileContext(nc) as tc, ExitStack() as ctx:
        const = ctx.enter_context(tc.tile_pool(name="const", bufs=1))
        persist = ctx.enter_context(tc.tile_pool(name="persist", bufs=1))
        dram = ctx.enter_context(tc.tile_pool(name="dram", bufs=1, space="DRAM"))
        ps_s = ctx.enter_context(
            tc.tile_pool(name="ps_s", bufs=3, space="PSUM")
        )

        cst_sb = const.tile([128, NCST], F32)
        nc.sync.dma_start(out=cst_sb[:, :], in_=cst[:, :])
        sc_sb = cst_sb[:, 0:1]
        bq_sb = cst_sb[:, 1 : 1 + CT]
        bk_sb = cst_sb[:, 1 + CT : 1 + 2 * CT]
        bv_sb = cst_sb[:, 1 + 2 * CT : 1 + 2 * CT + DIM]

        ones_w = const.tile([128, 128], BF16)
        nc.vector.memset(ones_w[:, :], 1.0)
        ones_m = const.tile([128, 512], BF16)
        nc.vector.memset(ones_m[:, :], 1.0)

        # collective bounce buffers (2 chunks each for K and V)
        kb_in = [dram.tile([512, QLEN], BF16, name=f"kbi{i}") for i in range(2)]
        kb_out = [dram.tile([1024, QLEN], BF16, name=f"kbo{i}") for i in range(2)]
        vb_in = [dram.tile([512, DIM], BF16, name=f"vbi{i}") for i in range(2)]
        vb_out = [dram.tile([1024, DIM], BF16, name=f"vbo{i}") for i in range(2)]

        # staging for locally-projected K/V halves (released after bounce DMA)
        stg = tc.alloc_tile_pool(name="stg", bufs=1)
        k_loc = [stg.tile([128, QLEN], BF16, name=f"kl{c}") for c in range(CT)]
        v_loc = [stg.tile([128, DIM], BF16, name=f"vl{k}") for k in range(LKT)]

        # input images; pools release LIFO (wk after K, wq after Q, wv then xq after V)
        xqp = tc.alloc_tile_pool(name="xqp", bufs=1)
        xq_im = [xqp.tile([128, DT * QLEN // 2], BF16, name=f"xq{h}") for h in range(2)]
        wvp = tc.alloc_tile_pool(name="wvp", bufs=1)
        wv_im = [wvp.tile([128, DT * DIM // 2], BF16, name=f"wv{h}") for h in range(2)]
        wqp = tc.alloc_tile_pool(name="wqp", bufs=1)
        wq_im = [wqp.tile([128, DT * DIM // 2], BF16, name=f"wq{h}") for h in range(2)]
        wkp = tc.alloc_tile_pool(name="wkp", bufs=1)
        wk_im = [wkp.tile([128, DT * DIM // 2], BF16, name=f"wk{h}") for h in range(2)]

        HW_ = DT * DIM // 2  # columns per half-image (4096)

        nc.sync.dma_start(out=wk_im[0][:, :], in_=wki[:, 0:HW_])
        nc.sync.dma_start(out=xq_im[0][:, :], in_=xqi[:, 0:HW_])
        nc.sync.dma_start(out=wk_im[1][:, :], in_=wki[:, HW_:])
        nc.sync.dma_start(out=xq_im[1][:, :], in_=xqi[:, HW_:])
        nc.sync.dma_start(out=wq_im[0][:, :], in_=wqi[:, 0:HW_])
        nc.sync.dma_start(out=wq_im[1][:, :], in_=wqi[:, HW_:])
        nc.sync.dma_start(out=wv_im[0][:, :], in_=wvi[:, 0:HW_])
        nc.sync.dma_start(out=wv_im[1][:, :], in_=wvi[:, HW_:])

        def wsl(im, d, lo, hi):
            h, dd = divmod(d, DT // 2)
            return im[h][:, dd * DIM + lo : dd * DIM + hi]

        def qsl(d, lo, hi):
            h, dd = divmod(d, DT // 2)
            return xq_im[h][:, dd * QLEN + lo : dd * QLEN + hi]

        # Warm the PE clock gate (HAM) during the initial input-DMA wait.
        with tc.psum_pool(name="ps_w", bufs=1) as ps_w:
            warm = ps_w.tile([128, 512], F32, name="warm")
            for i in range(12):
                nc.tensor.matmul(
                    warm[:, :], ones_w[:, :], ones_m[:, :], start=(i == 0), stop=(i == 11)
                )

        # persistent operands of the attention phase
        q_sb = [persist.tile([128, QLEN], BF16, name=f"q{i}") for i in range(CT)]
        k_full = [persist.tile([128, S], BF16, name=f"k{i}") for i in range(CT)]
        v_full = [persist.tile([128, DIM], BF16, name=f"v{i}") for i in range(KT)]

        # ---- Phase 1a: local K^T chunk-wise, AllGather per chunk ----
        with nc.named_scope("proj_k"):
            for i in range(2):
                for c in range(4 * i, 4 * i + 4):
                    pss = [ps_s.tile([128, 512], F32, tag="ps", name="psk") for _ in range(2)]
                    for d in range(DT):
                        for n in range(2):
                            nc.tensor.matmul(
                                pss[n][:, :],
                                wsl(wk_im, d, c * 128, (c + 1) * 128),
                                qsl(d, n * 512, (n + 1) * 512),
                                start=(d == 0),
                                stop=(d == DT - 1),
                            )
                    for n in range(2):
                        nc.scalar.activation(
                            k_loc[c][:, n * 512 : (n + 1) * 512],
                            pss[n][:, :],
                            Id,
                            bias=bk_sb[:, c : c + 1],
                        )
                    nc.gpsimd.dma_start(
                        out=kb_in[i][(c - 4 * i) * 128 : (c - 4 * i + 1) * 128, :],
                        in_=k_loc[c][:, :],
                    )
                nc.gpsimd.collective_compute(
                    "AllGather",
                    mybir.AluOpType.bypass,
                    replica_groups=GROUPS,
                    ins=[kb_in[i].opt()],
                    outs=[kb_out[i].opt()],
                )

        wkp.release()

        # ---- Phase 1b: Q^T = Wq^T.T @ X^T_local  (+bq) ----
        with nc.named_scope("proj_q"):
            for c in range(CT):
                pss = [ps_s.tile([128, 512], F32, tag="ps", name="psq") for _ in range(2)]
                for d in range(DT):
                    for n in range(2):
                        nc.tensor.matmul(
                            pss[n][:, :],
                            wsl(wq_im, d, c * 128, (c + 1) * 128),
                            qsl(d, n * 512, (n + 1) * 512),
                            start=(d == 0),
                            stop=(d == DT - 1),
                        )
                for n in range(2):
                    nc.scalar.activation(
                        q_sb[c][:, n * 512 : (n + 1) * 512],
                        pss[n][:, :],
                        Id,
                        bias=bq_sb[:, c : c + 1],
                    )

        wqp.release()

        # ---- Phase 1c: local V chunk-wise, AllGather per chunk ----
        with nc.named_scope("proj_v"):
            for i in range(2):
                for kk in range(4 * i, 4 * i + 4):
                    pss = [ps_s.tile([128, 512], F32, tag="ps", name="psv") for _ in range(2)]
                    for d in range(DT):
                        for n in range(2):
                            nc.tensor.matmul(
                                pss[n][:, :],
                                qsl(d, kk * 128, (kk + 1) * 128),
                                wsl(wv_im, d, n * 512, (n + 1) * 512),
                                start=(d == 0),
                                stop=(d == DT - 1),
                            )
                    for n in range(2):
                        nc.vector.tensor_add(
                            v_loc[kk][:, n * 512 : (n + 1) * 512],
                            pss[n][:, :],
                            bv_sb[:, n * 512 : (n + 1) * 512],
                        )
                    nc.gpsimd.dma_start(
                        out=vb_in[i][(kk - 4 * i) * 128 : (kk - 4 * i + 1) * 128, :],
                        in_=v_loc[kk][:, :],
                    )
                nc.gpsimd.collective_compute(
                    "AllGather",
                    mybir.AluOpType.bypass,
                    replica_groups=GROUPS,
                    ins=[vb_in[i].opt()],
                    outs=[vb_out[i].opt()],
                )

        wvp.release()
        xqp.release()

        # ---- gather readbacks: rank order == global key order on both
        # cores of a pair, so the indexing below is parity-free ----
        with nc.named_scope("gather_rd"):
            for i in range(2):
                for i2 in range(4):
                    c = 4 * i + i2
                    nc.sync.dma_start(
                        out=k_full[c][:, 0:QLEN],
                        in_=kb_out[i][i2 * 128 : (i2 + 1) * 128, :],
                    )
                    nc.sync.dma_start(
                        out=k_full[c][:, QLEN:S],
                        in_=kb_out[i][512 + i2 * 128 : 512 + (i2 + 1) * 128, :],
                    )
            for i in range(2):
                for i2 in range(4):
                    nc.sync.dma_start(
                        out=v_full[4 * i + i2][:, :],
                        in_=vb_out[i][i2 * 128 : (i2 + 1) * 128, :],
                    )
                    nc.sync.dma_start(
                        out=v_full[8 + 4 * i + i2][:, :],
                        in_=vb_out[i][512 + i2 * 128 : 512 + (i2 + 1) * 128, :],
                    )
        stg.release()

        # ---- Phase 2: attention ----
        lnsc_sb = const.tile([128, 1], F32)
        nc.scalar.activation(lnsc_sb[:, :], sc_sb, Ln)

        with (
            tc.tile_pool(name="pP", bufs=1) as pP,
            tc.tile_pool(name="ps_r", bufs=2, space="PSUM") as ps_r,
            tc.tile_pool(name="ps_u", bufs=3, space="PSUM") as ps_u,
            tc.tile_pool(name="small", bufs=8) as small,
            tc.tile_pool(name="tmp2", bufs=3) as tmp2,
            tc.tile_pool(name="ostage", bufs=4) as ostage,
        ):
            p_sb = [
                [
                    [pP.tile([128, 512], BF16, name=f"p{qc}_{j}_{k}") for k in range(KT)]
                    for j in range(2)
                ]
                for qc in range(NQC)
            ]
            bcs = [[None, None] for _ in range(NQC)]

            # scores + row-sums for all chunks first
            for qc in range(NQC):
                scope_s = nc.enter_named_scope(f"attn_s{qc}", False)
                for j in range(2):
                    r_ps = ps_r.tile([128, 512], F32, tag="r", name=f"r{qc}{j}")
                    for k in range(KT):
                        ps = ps_s.tile([128, 512], F32, tag="ps", name="pss")
                        for ci in range(4):
                            c = 4 * j + ci
                            nc.tensor.matmul(
                                ps[:, :],
                                k_full[c][:, k * 128 : (k + 1) * 128],
                                q_sb[c][:, qc * 512 : (qc + 1) * 512],
                                start=(ci == 0),
                                stop=(ci == 3),
                            )
                        nc.scalar.activation(
                            p_sb[qc][j][k][:, :], ps[:, :], Exp, scale=SCALE
                        )
                        nc.tensor.matmul(
                            r_ps[:, :],
                            ones_w[:, :],
                            p_sb[qc][j][k][:, :],
                            start=(k == 0),
                            stop=(k == KT - 1),
                        )
                    lnr = tmp2.tile([128, 512], F32, tag="lnr", name="lnr")
                    nc.scalar.activation(lnr[:, :], r_ps[:, :], Ln)
                    bc = small.tile([128, 512], BF16, tag=f"bc{qc}{j}", name=f"bc{qc}{j}")
                    if j == 0:
                        nc.scalar.activation(bc[:, :], lnr[:, :], Exp, scale=-1.0)
                    else:
                        nc.scalar.activation(
                            bc[:, :], lnr[:, :], Exp, scale=-1.0, bias=lnsc_sb[:, :]
                        )
                    bcs[qc][j] = bc
                nc.leave_named_scope(f"attn_s{qc}", scope_s[0], False)

            for qc in range(NQC):
                # A^T[k] = P1[k]*bc1 - P2[k]*bc2s  (in place into p_sb[qc][1])
                scope_a = nc.enter_named_scope(f"attn_a{qc}", False)
                for k in range(KT):
                    t2 = tmp2.tile([128, 512], BF16, tag="t2", name="t2")
                    nc.vector.tensor_mul(t2[:, :], p_sb[qc][0][k][:, :], bcs[qc][0][:, :])
                    nc.vector.tensor_mul(
                        p_sb[qc][1][k][:, :], p_sb[qc][1][k][:, :], bcs[qc][1][:, :]
                    )
                    nc.vector.tensor_sub(
                        p_sb[qc][1][k][:, :], t2[:, :], p_sb[qc][1][k][:, :]
                    )
                nc.leave_named_scope(f"attn_a{qc}", scope_a[0], False)

                # out rows = A^T.T @ V ; per-(t,n) psum groups, DMA on gpsimd
                scope_u = nc.enter_named_scope(f"attn_u{qc}", False)
                for t in range(4):
                    for n in range(2):
                        u = ps_u.tile([128, 512], F32, tag="u", name="u")
                        for k in range(KT):
                            nc.tensor.matmul(
                                u[:, :],
                                p_sb[qc][1][k][:, t * 128 : (t + 1) * 128],
                                v_full[k][:, n * 512 : (n + 1) * 512],
                                start=(k == 0),
                                stop=(k == KT - 1),
                            )
                        o = ostage.tile([128, 512], F32, tag="o", name="o")
                        if n == 0:
                            nc.scalar.copy(o[:, :], u[:, :])
                        else:
                            nc.vector.tensor_copy(o[:, :], u[:, :])
                        nc.gpsimd.dma_start(
                            out=outp[
                                qc * 128 : (qc + 1) * 128,
                                t * 1024 + n * 512 : t * 1024 + (n + 1) * 512,
                            ],
                            in_=o[:, :],
                        )
                nc.leave_named_scope(f"attn_u{qc}", scope_u[0], False)

    return nc


_NC_CACHE = None


def _get_nc():
    global _NC_CACHE
    if _NC_CACHE is None:
        nc = _build_bass()
        fixed = _split_waits(bass.Bass.to_json_bytes(nc))
        nc.to_json_bytes = lambda: fixed
        _NC_CACHE = nc
    return _NC_CACHE


def _img(a32):
    """[1024, W] fp32 -> [128, 8*W] bf16 SBUF image (d-major blocks)."""
    W = a32.shape[1]
    return np.ascontiguousarray(
        a32.reshape(DT, 128, W).transpose(1, 0, 2).reshape(128, DT * W)
    ).astype(ml_dtypes.bfloat16)


def kernel(hidden_states, W_q, b_q, W_k, b_k, W_v, b_v, scalar):
    global LAST_RESULTS
    X = np.asarray(hidden_states, np.float32)
    wq_img = _img(np.ascontiguousarray(np.asarray(W_q, np.float32).T))
    wk_img = _img(np.ascontiguousarray(np.asarray(W_k, np.float32).T))
    wv_img = _img(np.ascontiguousarray(np.asarray(W_v, np.float32).T))

    cst = np.empty((128, NCST), np.float32)
    cst[:, 0] = np.asarray(scalar, np.float32).reshape(-1)[0]
    cst[:, 1 : 1 + CT] = np.asarray(b_q, np.float32).reshape(CT, 128).T
    cst[:, 1 + CT : 1 + 2 * CT] = np.asarray(b_k, np.float32).reshape(CT, 128).T
    cst[:, 1 + 2 * CT :] = np.broadcast_to(np.asarray(b_v, np.float32), (128, DIM))

    in_maps = []
    for core in range(NCORES):
        b, h = core // 2, core % 2
        xq_img = _img(
            np.ascontiguousarray(X[b].T[:, h * QLEN : (h + 1) * QLEN])
        )
        in_maps.append(
            {
                "xqi": xq_img,
                "wqi": wq_img,
                "wki": wk_img,
                "wvi": wv_img,
                "cst": cst,
            }
        )

    nc = _get_nc()
    res = run_bass_kernel_spmd(
        nc,
        in_maps,
        list(range(NCORES)),
        trace=TRACE,
    )
    LAST_RESULTS = res

    out = np.empty((B, S, DIM), np.float32)
    for core in range(NCORES):
        b, h = core // 2, core % 2
        # device layout [qc*128+p, t*1024 + n*512 + cc] -> [qc*512+t*128+p, :]
        dev = res.results[core]["out"].reshape(NQC, 128, 4, DIM)
        out[b, h * QLEN : (h + 1) * QLEN, :] = (
            dev.transpose(0, 2, 1, 3).reshape(QLEN, DIM)
        )
    return out


if __name__ == "__main__":
    import reference

    inputs = {k: np.asarray(v) for k, v in reference.setup_inputs().items()}
    got = kernel(**inputs)
    print("kernel output", got.shape, got.dtype)
